# revision 48
# baseline (speedup 1.0000x reference)
"""BevFeatureEncoder on 8 Trainium2 NeuronCores.

Strategy (data-parallel over BEV grid slabs):
  - The 2*480*360 BEV cells are split into 8 contiguous ranges of 43200
    cells. Points are routed on host to the core owning their cell, so
    the segment_max reduction is fully local to each core.
  - On host (integer indexing only), each core's occupied cells are
    grouped by point count, counts padded up to k by duplicating points
    of the same cell (a no-op under max). Cells are processed in chunks
    of <=512; points are laid out so slot s of a chunk is a dense,
    contiguous block of points.
  - BN scale/bias are folded into the weights ON HOST; all matmuls are
    K=128, M=128, bf16 (uniform PE configuration: no weight-path
    stalls from contraction-depth or dtype switches; FWL fast loads).
  - Layer 1 uses a block-diagonal weight (two w1 copies): each moving
    column carries TWO points, halving mm1 columns.
  - Dummy warmup ops run during the initial DMA wait: an early ACT op
    forces the lazy ACT_TABLE_LOAD, and a 28-matmul PSUM accumulation
    group opens the HAM clock gate before the first real matmul.
  - Two chunks are compressed into one [128, c] PSUM tile (second wc
    copy shifted to output partitions 64-127), halving the final
    bias+relu ops and giving dense output DMA.
"""

import numpy as np
import ml_dtypes

import concourse.bacc as bacc
import concourse.bass as bass
import concourse.mybir as mybir
import concourse.tile as tile
from concourse import bass_utils

GX, GY = 480, 360
B = 2
EPS = 1e-5
N_CORES = 8
CELLS_PER_CORE = (B * GX * GY) // N_CORES  # 43200
CHUNK = 512
PAD = -1

F32 = mybir.dt.float32
BF16 = mybir.dt.bfloat16

Relu = mybir.ActivationFunctionType.Relu


# ---------------------------------------------------------------- host prep


def _prep_core(seg_local, lo_idx):
    """Group one core's occupied cells by padded point count."""
    order = np.argsort(seg_local, kind="stable")
    seg_sorted = seg_local[order]
    cells, starts, counts = np.unique(
        seg_sorted, return_index=True, return_counts=True
    )
    ks2 = 1 << (np.ceil(np.log2(np.maximum(counts, 1))).astype(np.int64))
    ks = np.where(counts <= 4, counts, np.maximum(ks2, 1)).astype(np.int64)
    out = {}
    for k in np.unique(ks):
        sel = np.nonzero(ks == k)[0]
        slots = np.empty((len(sel), int(k)), np.int64)
        for s in range(int(k)):
            slots[:, s] = order[starts[sel] + np.minimum(s, counts[sel] - 1)]
        out[int(k)] = (cells[sel].astype(np.int64), lo_idx[slots])
    return out


def _plan_items(chunk_plan):
    """Flat slot-item stream (ci, k, c, s, pt_off) and mm1 item pairs."""
    items = []
    pt = 0
    for ci, (k, c) in enumerate(chunk_plan):
        for s in range(k):
            items.append((ci, k, c, s, pt))
            pt += c
    # mm1 pairs: consecutive equal-width items share one block-diag matmul
    pairs = []
    i = 0
    while i < len(items):
        if i + 1 < len(items) and items[i + 1][2] == items[i][2]:
            pairs.append([items[i], items[i + 1]])
            i += 2
        else:
            pairs.append([items[i]])
            i += 1
    # quads: consecutive pairs share wide PSUM tiles; stage-2 tile (one
    # column per ITEM) <= 1024, stage-1 tile (one column per PAIR) <= 512
    quads = []
    i = 0
    while i < len(pairs):
        quad = [pairs[i]]
        w1 = pairs[i][0][2]
        w2 = len(pairs[i]) * pairs[i][0][2]
        i += 1
        while (i < len(pairs)
               and w2 + len(pairs[i]) * pairs[i][0][2] <= 1024
               and w1 + pairs[i][0][2] <= 512):
            quad.append(pairs[i])
            w1 += pairs[i][0][2]
            w2 += len(pairs[i]) * pairs[i][0][2]
            i += 1
        quads.append(quad)
    return items, pairs, quads


def _pair_chunks(chunk_plan):
    """Pair chunks (in completion order) for the 2-in-1 compression tile.

    Returns (fpairs, pair_of, pair_cells): fpairs is a list of
    (ci, cj_or_None, out_off, c); pair_of maps ci -> fpair index."""
    fpairs = []
    pair_of = {}
    off = 0
    pend = None  # (ci, c, k)
    for ci, (k, c) in enumerate(chunk_plan):
        if pend is not None and pend[1] == c:
            fp = (pend[0], ci, off, c)
            pair_of[pend[0]] = len(fpairs)
            pair_of[ci] = len(fpairs)
            fpairs.append(fp)
            off += c
            pend = None
        else:
            if pend is not None:
                fp = (pend[0], None, off, pend[1])
                pair_of[pend[0]] = len(fpairs)
                fpairs.append(fp)
                off += pend[1]
            pend = (ci, c, k)
    if pend is not None:
        fp = (pend[0], None, off, pend[1])
        pair_of[pend[0]] = len(fpairs)
        fpairs.append(fp)
        off += pend[1]
    return fpairs, pair_of, off


def _build_plan_and_data(voxels, coors):
    """Route points to cores, build the equalized chunk plan plus per-core
    device inputs (block-diag packed voxels, bf16) and placement tables."""
    seg = (
        coors[:, 0].astype(np.int64) * (GX * GY)
        + coors[:, 1].astype(np.int64) * GY
        + coors[:, 2].astype(np.int64)
    )
    core_of = seg // CELLS_PER_CORE
    per_core = []
    for c in range(N_CORES):
        idx = np.nonzero(core_of == c)[0]
        per_core.append(_prep_core(seg[idx] - c * CELLS_PER_CORE, idx))

    all_ks = sorted({k for g in per_core for k in g.keys()})
    raw_plan = []  # (k, c)
    for k in all_ks:
        n_max = max(len(g[k][0]) if k in g else 0 for g in per_core)
        n_pad = -(-n_max // 128) * 128
        while n_pad > 0:
            c = min(n_pad, CHUNK)
            if c == 384:
                raw_plan.append((k, 256))
                n_pad -= 256
                continue
            raw_plan.append((k, c))
            n_pad -= c
    # width-sorted: equal-width items adjacent so mm1 pairs/quads pack
    # fully; within the small tail width classes, deepest chains first
    chunk_plan = sorted(
        raw_plan,
        key=lambda kc: (-kc[1], kc[0] if kc[1] == 512 else -kc[0]))
    total_cells = sum(c for _, c in chunk_plan)
    items, pairs, quads = _plan_items(chunk_plan)
    vox_cols = sum(p[0][2] for p in pairs)

    vox_all = np.zeros((N_CORES, 128, vox_cols), ml_dtypes.bfloat16)
    rows_all = np.full((N_CORES, total_cells), PAD, np.int64)

    for core in range(N_CORES):
        groups = per_core[core]
        cell0 = 0
        used = {}
        src = {}
        for ci, (k, c) in enumerate(chunk_plan):
            cells, slots = groups.get(
                k, (np.zeros(0, np.int64), np.zeros((0, k), np.int64)))
            u = used.get(k, 0)
            batch_cells = cells[u : u + c]
            batch_slots = slots[u : u + c]
            used[k] = u + c
            nb = len(batch_cells)
            sl = np.zeros((c, k), np.int64)
            if nb:
                sl[:nb] = batch_slots
                sl[nb:] = batch_slots[0, 0]
            elif len(cells):
                sl[:] = slots[0, 0]
            for s in range(k):
                src[(ci, s)] = sl[:, s]
            rows_all[core, cell0 : cell0 + nb] = batch_cells
            cell0 += c
        col = 0
        vx = np.asarray(voxels, np.float32)
        for pr in pairs:
            c = pr[0][2]
            ia = src[(pr[0][0], pr[0][3])]
            vox_all[core, 0:4, col : col + c] = vx[ia].T
            if len(pr) == 2:
                ib = src[(pr[1][0], pr[1][3])]
                vox_all[core, 4:8, col : col + c] = vx[ib].T
            col += c
        assert col == vox_cols and cell0 == total_cells
    return chunk_plan, vox_cols, vox_all, rows_all


# ------------------------------------------------------------- bass program


def build_program(chunk_plan, vox_cols):
    fpairs, pair_of, pair_cells = _pair_chunks(chunk_plan)
    nc = bacc.Bacc("TRN2", target_bir_lowering=False, debug=False,
                   num_devices=N_CORES)

    vox = nc.dram_tensor("vox", [128, vox_cols], BF16,
                         kind="ExternalInput").ap()
    w_in = {}
    for name, shape, dt in [
        ("wpack", [128, 768], BF16), ("bpack", [128, 5], F32),
    ]:
        w_in[name] = nc.dram_tensor(name, shape, dt, kind="ExternalInput").ap()
    comp = nc.dram_tensor("comp", [128, pair_cells], BF16,
                          kind="ExternalOutput").ap()

    from contextlib import ExitStack
    with tile.TileContext(nc) as tc, ExitStack() as ctx:
        cpool = ctx.enter_context(tc.tile_pool(name="const", bufs=1))

        wpk = cpool.tile([128, 768], BF16, tag="wpack")
        nc.sync.dma_start(out=wpk[:], in_=w_in["wpack"])
        w1d = wpk[:, 0:128]
        w2e = wpk[:, 128:256]
        w2o = wpk[:, 256:384]
        w3a = wpk[:, 384:512]
        w3b = wpk[:, 512:640]
        wc0 = wpk[:, 640:704]
        wc1 = wpk[:, 704:768]
        bpk = cpool.tile([128, 5], F32, tag="bpack")
        nc.sync.dma_start(out=bpk[:], in_=w_in["bpack"])
        t1x = bpk[:, 0:1]
        t2 = bpk[:, 1:2]
        t3a = bpk[:, 2:3]
        t3b = bpk[:, 3:4]
        bcr = bpk[:, 4:5]

        sb = ctx.enter_context(tc.tile_pool(name="sb", bufs=8))
        scp = ctx.enter_context(tc.tile_pool(name="scp", bufs=3))
        vxp = ctx.enter_context(tc.tile_pool(name="vx", bufs=4))
        # PSUM (8 banks): p1 [128,<=512] x1, p2 [128,<=1024] x1,
        # psA/psB [128,<=1024] x1 each, pc [128,<=512] x1
        p1p = ctx.enter_context(tc.tile_pool(name="p1p", bufs=1, space="PSUM"))
        p2p = ctx.enter_context(tc.tile_pool(name="p2p", bufs=1, space="PSUM"))
        psA = ctx.enter_context(tc.tile_pool(name="psA", bufs=1, space="PSUM"))
        psB = ctx.enter_context(tc.tile_pool(name="psB", bufs=1, space="PSUM"))
        pcp = ctx.enter_context(tc.tile_pool(name="pcp", bufs=1, space="PSUM"))

        def ldw(w):
            pass

        def br_dve(out_ap, in_ap, bias_ap):
            nc.vector.tensor_scalar(
                out_ap, in_ap, bias_ap, 0.0,
                op0=mybir.AluOpType.add, op1=mybir.AluOpType.max)

        debt = {"act": 0.0, "dve": 0.0}

        def br_auto(out_ap, in_ap, bias_ap):
            c = in_ap.shape[-1]
            ca, cd = (c + 190) / 1.2, (c + 120) / 0.96
            if debt["act"] + ca <= debt["dve"] + cd:
                debt["act"] += ca
                nc.scalar.activation(out_ap, in_ap, Relu, bias=bias_ap,
                                     scale=1.0)
            else:
                debt["dve"] += cd
                br_dve(out_ap, in_ap, bias_ap)

        def dve_forced(c):
            debt["dve"] += (c + 120) / 0.96

        def br_split(out_ap, in_ap, bias_ap):
            # wide evac split across both engines: halves the PSUM
            # recycle latency at the cost of one extra op's overhead
            W = in_ap.shape[-1]
            h = W // 2
            debt["act"] += (h + 190) / 1.2
            nc.scalar.activation(out_ap[:, 0:h], in_ap[:, 0:h], Relu,
                                 bias=bias_ap, scale=1.0)
            debt["dve"] += (h + 120) / 0.96
            br_dve(out_ap[:, h:W], in_ap[:, h:W], bias_ap)

        def max_br_dve(out_ap, in_ap, bias_ap, acc_ap):
            nc.vector.scalar_tensor_tensor(
                out_ap, in_ap, bias_ap, acc_ap,
                op0=mybir.AluOpType.add, op1=mybir.AluOpType.max)

        # --- engine warmup: runs during the initial DMA wait ---
        # dummy ACT op forces the lazy ACT_TABLE_LOAD early; dummy matmul
        # chain keeps the PE busy so the HAM clock gate opens (2.4 GHz)
        # before the first real matmul; none of these touch DMA'd data
        dmy = cpool.tile([128, 128], BF16, tag="dmy")
        nc.vector.memset(dmy[:], 0.25)
        dbz = cpool.tile([128, 1], F32, tag="dbz")
        nc.vector.memset(dbz[:], 0.0)
        da = cpool.tile([128, 64], BF16, tag="da")
        nc.scalar.activation(da[:], dmy[:, 0:64], Relu, bias=dbz[:],
                             scale=1.0)
        dv = cpool.tile([128, 64], BF16, tag="dv")
        nc.vector.tensor_scalar(dv[:], dmy[:, 0:64], dbz[:], 0.0,
                                op0=mybir.AluOpType.add,
                                op1=mybir.AluOpType.max)
        pw = pcp.tile([128, 128], F32, tag="pc", space="PSUM",
                      name="warm")
        for wn in range(24):
            nc.tensor.matmul(pw[:], dmy[:], dmy[:], start=(wn == 0),
                             stop=(wn == 23))

        items, pairs, quads = _plan_items(chunk_plan)
        cell_off = []
        co = 0
        for ci, (k, c) in enumerate(chunk_plan):
            cell_off.append(co)
            co += c
        pair_off = {}
        po = 0
        for pi, pr in enumerate(pairs):
            pair_off[pi] = po
            po += pr[0][2]
        pair_idx = {id(pr): pi for pi, pr in enumerate(pairs)}

        # batched vox loads; first slab small so mm1 starts early
        vx_ap = {}
        batch = []
        bcols = 0
        slab_cap = [512]

        def flush_vox():
            nonlocal batch, bcols
            if not batch:
                return
            p0 = pair_off[batch[0]]
            vx = vxp.tile([128, bcols], BF16, tag="vx", name=f"vx{p0}")
            nc.sync.dma_start(out=vx[:], in_=vox[:, p0 : p0 + bcols])
            for pi in batch:
                rel = pair_off[pi] - p0
                vx_ap[pi] = vx[:, rel : rel + pairs[pi][0][2]]
            batch = []
            bcols = 0
            slab_cap[0] = 4096

        for pi, pr in enumerate(pairs):
            if bcols + pr[0][2] > slab_cap[0]:
                flush_vox()
            batch.append(pi)
            bcols += pr[0][2]
        flush_vox()

        # chunk state
        accA = {}
        accB = {}
        uaccA = {}
        uaccB = {}
        ucnt = {}
        done_chunks = set()
        fin_q = []

        def chunk_done(ci, k, c):
            done_chunks.add(ci)
            fpi = pair_of[ci]
            ca, cb, off, cw = fpairs[fpi]
            if cb is None or (ca in done_chunks and cb in done_chunks):
                fin_q.append((fpi,))

        def finalize_pair(fpi):
            ca, cb, off, c = fpairs[fpi]
            pc = pcp.tile([128, c], F32, tag="pc", space="PSUM",
                          name=f"pc{fpi}")
            nc.tensor.matmul(pc[0:64, :], wc0, accA.pop(ca),
                             start=True, stop=False)
            if cb is not None:
                nc.tensor.matmul(pc[64:128, :], wc0, accA.pop(cb),
                                 start=True, stop=False)
            nc.tensor.matmul(pc[0:64, :], wc1, accB.pop(ca),
                             start=False, stop=True)
            if cb is not None:
                nc.tensor.matmul(pc[64:128, :], wc1, accB.pop(cb),
                                 start=False, stop=True)
            sc = scp.tile([128, c], BF16, tag="sc", name=f"sc{fpi}")
            br_auto(sc[:], pc[:], bcr)
            q = nc.sync if fpi % 2 else nc.gpsimd
            q.dma_start(out=comp[:, off : off + c], in_=sc[:])

        def stage1(quad):
            """block-diag mm1 per pair -> one wide p1 -> h1w (bf16)."""
            W = sum(pr[0][2] for pr in quad)
            nm = f"{quad[0][0][0]}_{quad[0][0][3]}"
            p1 = p1p.tile([128, W], F32, tag="p1", space="PSUM",
                          name=f"p1_{nm}")
            ldw(w1d)
            o = 0
            for pr in quad:
                c = pr[0][2]
                nc.tensor.matmul(p1[:, o : o + c], w1d,
                                 vx_ap[pair_idx[id(pr)]],
                                 start=True, stop=True)
                o += c
            h1 = sb.tile([128, W], BF16, tag="h1", name=f"h1_{nm}")
            br_auto(h1[:], p1[:], t1x)
            return h1

        def stage2(quad, h1):
            """mm2 all-evens then all-odds -> one wide p2 -> h2w (bf16)."""
            W = sum(len(pr) * pr[0][2] for pr in quad)
            nm = f"{quad[0][0][0]}_{quad[0][0][3]}"
            p2 = p2p.tile([128, W], F32, tag="p2", space="PSUM",
                          name=f"p2_{nm}")
            parts = []
            ne = sum(pr[0][2] for pr in quad)
            oe, oo, ho = 0, ne, 0
            ldw(w2e)
            for pr in quad:
                c = pr[0][2]
                nc.tensor.matmul(p2[:, oe : oe + c], w2e,
                                 h1[:, ho : ho + c], start=True, stop=True)
                parts.append((pr[0], oe))
                oe += c
                ho += c
            ldw(w2o)
            ho = 0
            for pr in quad:
                c = pr[0][2]
                if len(pr) == 2:
                    nc.tensor.matmul(p2[:, oo : oo + c], w2o,
                                     h1[:, ho : ho + c],
                                     start=True, stop=True)
                    parts.append((pr[1], oo))
                    oo += c
                ho += c
            h2 = sb.tile([128, W], BF16, tag="h2", name=f"h2_{nm}")
            br_auto(h2[:], p2[:], t2)
            # restore stream order for stage3 (chunk slots in order)
            parts.sort(key=lambda po_: po_[0][4])
            return h2, parts

        def stage3_duo(todo):
            """mm3a/b for <=2 items sharing wide psA/psB tiles."""
            W = sum(it[2] for it, _ in todo)
            nm = f"{todo[0][0][0]}_{todo[0][0][3]}"
            pA = psA.tile([128, W], F32, tag="pA", space="PSUM",
                          name=f"pA_{nm}")
            pB = psB.tile([128, W], F32, tag="pB", space="PSUM",
                          name=f"pB_{nm}")
            ldw(w3a)
            o = 0
            for it, h2_ap in todo:
                nc.tensor.matmul(pA[:, o : o + it[2]], w3a, h2_ap,
                                 start=True, stop=True)
                o += it[2]
            ldw(w3b)
            o = 0
            for it, h2_ap in todo:
                nc.tensor.matmul(pB[:, o : o + it[2]], w3b, h2_ap,
                                 start=True, stop=True)
                o += it[2]
            # evacuation
            if (len(todo) == 2 and todo[0][0][1] == 1 and todo[1][0][1] == 1):
                # two single-slot chunks: one wide bias+relu per half
                c0 = todo[0][0][2]
                ciX, ciY = todo[0][0][0], todo[1][0][0]
                aA = sb.tile([128, W], BF16, tag="accrA", name=f"aA_{ciX}w")
                aB = sb.tile([128, W], BF16, tag="accrB", name=f"aB_{ciX}w")
                br_auto(aA[:], pA[:], t3a)
                br_auto(aB[:], pB[:], t3b)
                accA[ciX], accB[ciX] = aA[:, 0:c0], aB[:, 0:c0]
                accA[ciY], accB[ciY] = aA[:, c0:W], aB[:, c0:W]
                for it, _ in todo:
                    chunk_done(it[0], it[1], it[2])
            else:
                o = 0
                for it, _ in todo:
                    slot_ops(it, pA[:, o : o + it[2]], pB[:, o : o + it[2]])
                    o += it[2]

        def slot_ops(it, pA_ap, pB_ap):
            ci, k, c, s, po_ = it
            if k == 1:
                aA = sb.tile([128, c], BF16, tag="accrA", name=f"aA_{ci}")
                br_auto(aA[:], pA_ap, t3a)
                aB = sb.tile([128, c], BF16, tag="accrB", name=f"aB_{ci}")
                br_auto(aB[:], pB_ap, t3b)
                accA[ci], accB[ci] = aA[:], aB[:]
            elif s == 0:
                aA = [sb.tile([128, c], BF16, tag="accrA",
                              name=f"aA_{ci}_{j}") for j in range(2)]
                aB = [sb.tile([128, c], BF16, tag="accrB",
                              name=f"aB_{ci}_{j}") for j in range(2)]
                br_auto(aA[0][:], pA_ap, t3a)
                br_auto(aB[0][:], pB_ap, t3b)
                accA[ci] = aA
                accB[ci] = aB
                accA[f"n{ci}"] = 0
                accB[f"n{ci}"] = 0
            else:
                nA = accA[f"n{ci}"]
                max_br_dve(accA[ci][1 - nA][:], pA_ap, t3a,
                           accA[ci][nA][:])
                accA[f"n{ci}"] = 1 - nA
                nB = accB[f"n{ci}"]
                max_br_dve(accB[ci][1 - nB][:], pB_ap, t3b,
                           accB[ci][nB][:])
                accB[f"n{ci}"] = 1 - nB
                dve_forced(c)
                dve_forced(c)
            if s == k - 1:
                if k > 1:
                    nA, nB = accA.pop(f"n{ci}"), accB.pop(f"n{ci}")
                    accA[ci] = accA[ci][nA][:]
                    accB[ci] = accB[ci][nB][:]
                chunk_done(ci, k, c)

        # software-pipelined emission over quads
        h1q = []
        h2q = []
        n_quads = len(quads)
        for t in range(n_quads + 3):
            ready_fins = list(fin_q)
            fin_q.clear()
            if t < n_quads:
                h1q.append((quads[t], stage1(quads[t])))
            if t >= 1 and h1q:
                qd, h1 = h1q.pop(0)
                h2q.append(stage2(qd, h1))
            if t >= 2 and h2q:
                h2, parts = h2q.pop(0)
                j = 0
                while j < len(parts):
                    duo = [parts[j]]
                    it0 = parts[j][0]
                    take2 = False
                    if j + 1 < len(parts):
                        it1 = parts[j + 1][0]
                        if it0[1] == 1 and it1[1] == 1:
                            take2 = (it0[2] == 512
                                     or it0[2] + it1[2] <= 512)
                        elif it0[0] == it1[0]:
                            take2 = True
                    if take2:
                        duo.append(parts[j + 1])
                        j += 2
                    else:
                        j += 1
                    stage3_duo([(it, h2[:, o : o + it[2]]) for it, o in duo])
            for f in ready_fins:
                finalize_pair(*f)

    nc.compile()
    return nc


# ------------------------------------------------------------------ driver

_CACHE = {}


def kernel(voxels, coors, batch_size, w1, g1, b1, m1, v1,
           w2, g2, b2, m2, v2, w3, g3, b3, m3, v3, wc, bc,
           _trace=False):
    voxels = np.asarray(voxels, np.float32)
    coors = np.asarray(coors, np.int32)
    chunk_plan, vox_cols, vox_all, rows_all = _build_plan_and_data(
        voxels, coors)

    key = tuple(chunk_plan)
    if key not in _CACHE:
        _CACHE[key] = build_program(chunk_plan, vox_cols)
    nc = _CACHE[key]

    def fold(w, g, b, m, v):
        s = np.asarray(g, np.float32) / np.sqrt(
            np.asarray(v, np.float32) + np.float32(EPS))
        wf = np.asarray(w, np.float32) * s[None, :]
        t = np.asarray(b, np.float32) - np.asarray(m, np.float32) * s
        return wf, t

    w1f, t1 = fold(w1, g1, b1, m1, v1)
    w2f, t2 = fold(w2, g2, b2, m2, v2)
    w3f, t3 = fold(w3, g3, b3, m3, v3)

    bf = ml_dtypes.bfloat16
    w1dm = np.zeros((128, 128), np.float32)
    w1dm[0:4, 0:64] = w1f
    w1dm[4:8, 64:128] = w1f
    w2em = np.zeros((128, 128), np.float32)
    w2em[0:64, :] = w2f
    w2om = np.zeros((128, 128), np.float32)
    w2om[64:128, :] = w2f
    wcf = np.asarray(wc, np.float32)
    wc0p = np.zeros((128, 128), np.float32)
    wc0p[:, 0:64] = wcf[0:128]
    wc1p = np.zeros((128, 128), np.float32)
    wc1p[:, 0:64] = wcf[128:256]
    wc0q = np.zeros((128, 128), np.float32)
    wc0q[:, 64:128] = wcf[0:128]
    wc1q = np.zeros((128, 128), np.float32)
    wc1q[:, 64:128] = wcf[128:256]
    t1xm = np.concatenate([t1, t1]).astype(np.float32)
    bcf_ = np.asarray(bc, np.float32)
    bcu_ = bcf_ + t3.astype(np.float32) @ wcf
    bcrm = np.concatenate([bcf_, bcf_]).astype(np.float32)
    bcum = np.concatenate([bcu_, bcu_]).astype(np.float32)

    wpack = np.zeros((128, 768), np.float32)
    wpack[:, 0:128] = w1dm
    wpack[:, 128:256] = w2em
    wpack[:, 256:384] = w2om
    wpack[:, 384:512] = np.ascontiguousarray(w3f[:, 0:128])
    wpack[:, 512:640] = np.ascontiguousarray(w3f[:, 128:256])
    wpack[:, 640:704] = wcf[0:128]
    wpack[:, 704:768] = wcf[128:256]
    bpack = np.zeros((128, 5), np.float32)
    bpack[:, 0] = t1xm
    bpack[:, 1] = t2
    bpack[:, 2] = t3[0:128]
    bpack[:, 3] = t3[128:256]
    bpack[:, 4] = bcrm
    weights = {
        "wpack": wpack.astype(bf),
        "bpack": bpack,
    }
    in_maps = [{"vox": vox_all[c], **weights} for c in range(N_CORES)]
    res = bass_utils.run_bass_kernel_spmd(
        nc, in_maps, core_ids=list(range(N_CORES)), trace=_trace)

    fpairs, pair_of, pair_cells = _pair_chunks(chunk_plan)
    cell_off = []
    co = 0
    for ci, (k, c) in enumerate(chunk_plan):
        cell_off.append(co)
        co += c
    total_cells = co

    out = np.zeros((B, 64, GX * GY), np.float32)
    for c in range(N_CORES):
        cp = res.results[c]["comp"].astype(np.float32)  # [128, pair_cells]
        cm = np.zeros((64, total_cells), np.float32)
        for ca, cb, off, cw in fpairs:
            cm[:, cell_off[ca] : cell_off[ca] + cw] = cp[0:64, off : off + cw]
            if cb is not None:
                cm[:, cell_off[cb] : cell_off[cb] + cw] = (
                    cp[64:128, off : off + cw])
        rows = rows_all[c]
        real = rows != PAD
        gcell = rows[real] + c * CELLS_PER_CORE
        b_core = c // (N_CORES // B)
        xy = gcell - b_core * (GX * GY)
        out[b_core][:, xy] = cm[:, real]
    out = out.reshape(B, 64, GX, GY)
    if _trace:
        return out, res
    return out


# revision 49
# speedup vs baseline: 1.0018x; 1.0018x over previous
"""BevFeatureEncoder on 8 Trainium2 NeuronCores.

Strategy (data-parallel over BEV grid slabs):
  - The 2*480*360 BEV cells are split into 8 contiguous ranges of 43200
    cells. Points are routed on host to the core owning their cell, so
    the segment_max reduction is fully local to each core.
  - On host (integer indexing only), each core's occupied cells are
    grouped by point count, counts padded up to k by duplicating points
    of the same cell (a no-op under max). Cells are processed in chunks
    of <=512; points are laid out so slot s of a chunk is a dense,
    contiguous block of points.
  - BN scale/bias are folded into the weights ON HOST; all matmuls are
    K=128, M=128, bf16 (uniform PE configuration: no weight-path
    stalls from contraction-depth or dtype switches; FWL fast loads).
  - Layer 1 uses a block-diagonal weight (two w1 copies): each moving
    column carries TWO points, halving mm1 columns.
  - Dummy warmup ops run during the initial DMA wait: an early ACT op
    forces the lazy ACT_TABLE_LOAD, and a 28-matmul PSUM accumulation
    group opens the HAM clock gate before the first real matmul.
  - Two chunks are compressed into one [128, c] PSUM tile (second wc
    copy shifted to output partitions 64-127), halving the final
    bias+relu ops and giving dense output DMA.
"""

import numpy as np
import ml_dtypes

import concourse.bacc as bacc
import concourse.bass as bass
import concourse.mybir as mybir
import concourse.tile as tile
from concourse import bass_utils

GX, GY = 480, 360
B = 2
EPS = 1e-5
N_CORES = 8
CELLS_PER_CORE = (B * GX * GY) // N_CORES  # 43200
CHUNK = 512
PAD = -1

F32 = mybir.dt.float32
BF16 = mybir.dt.bfloat16

Relu = mybir.ActivationFunctionType.Relu


# ---------------------------------------------------------------- host prep


def _prep_core(seg_local, lo_idx):
    """Group one core's occupied cells by padded point count."""
    order = np.argsort(seg_local, kind="stable")
    seg_sorted = seg_local[order]
    cells, starts, counts = np.unique(
        seg_sorted, return_index=True, return_counts=True
    )
    ks2 = 1 << (np.ceil(np.log2(np.maximum(counts, 1))).astype(np.int64))
    ks = np.where(counts <= 4, counts, np.maximum(ks2, 1)).astype(np.int64)
    out = {}
    for k in np.unique(ks):
        sel = np.nonzero(ks == k)[0]
        slots = np.empty((len(sel), int(k)), np.int64)
        for s in range(int(k)):
            slots[:, s] = order[starts[sel] + np.minimum(s, counts[sel] - 1)]
        out[int(k)] = (cells[sel].astype(np.int64), lo_idx[slots])
    return out


def _plan_items(chunk_plan):
    """Flat slot-item stream (ci, k, c, s, pt_off) and mm1 item pairs."""
    items = []
    pt = 0
    for ci, (k, c) in enumerate(chunk_plan):
        for s in range(k):
            items.append((ci, k, c, s, pt))
            pt += c
    # mm1 pairs: consecutive equal-width items share one block-diag matmul
    pairs = []
    i = 0
    while i < len(items):
        if i + 1 < len(items) and items[i + 1][2] == items[i][2]:
            pairs.append([items[i], items[i + 1]])
            i += 2
        else:
            pairs.append([items[i]])
            i += 1
    # quads: consecutive pairs share wide PSUM tiles; stage-2 tile (one
    # column per ITEM) <= 1024, stage-1 tile (one column per PAIR) <= 512
    quads = []
    i = 0
    while i < len(pairs):
        quad = [pairs[i]]
        w1 = pairs[i][0][2]
        w2 = len(pairs[i]) * pairs[i][0][2]
        i += 1
        while (i < len(pairs)
               and w2 + len(pairs[i]) * pairs[i][0][2] <= 1024
               and w1 + pairs[i][0][2] <= 512):
            quad.append(pairs[i])
            w1 += pairs[i][0][2]
            w2 += len(pairs[i]) * pairs[i][0][2]
            i += 1
        quads.append(quad)
    return items, pairs, quads


def _pair_chunks(chunk_plan):
    """Pair chunks (in completion order) for the 2-in-1 compression tile.

    Returns (fpairs, pair_of, pair_cells): fpairs is a list of
    (ci, cj_or_None, out_off, c); pair_of maps ci -> fpair index."""
    fpairs = []
    pair_of = {}
    off = 0
    pend = None  # (ci, c, k)
    for ci, (k, c) in enumerate(chunk_plan):
        if pend is not None and pend[1] == c:
            fp = (pend[0], ci, off, c)
            pair_of[pend[0]] = len(fpairs)
            pair_of[ci] = len(fpairs)
            fpairs.append(fp)
            off += c
            pend = None
        else:
            if pend is not None:
                fp = (pend[0], None, off, pend[1])
                pair_of[pend[0]] = len(fpairs)
                fpairs.append(fp)
                off += pend[1]
            pend = (ci, c, k)
    if pend is not None:
        fp = (pend[0], None, off, pend[1])
        pair_of[pend[0]] = len(fpairs)
        fpairs.append(fp)
        off += pend[1]
    return fpairs, pair_of, off


def _build_plan_and_data(voxels, coors):
    """Route points to cores, build the equalized chunk plan plus per-core
    device inputs (block-diag packed voxels, bf16) and placement tables."""
    seg = (
        coors[:, 0].astype(np.int64) * (GX * GY)
        + coors[:, 1].astype(np.int64) * GY
        + coors[:, 2].astype(np.int64)
    )
    core_of = seg // CELLS_PER_CORE
    per_core = []
    for c in range(N_CORES):
        idx = np.nonzero(core_of == c)[0]
        per_core.append(_prep_core(seg[idx] - c * CELLS_PER_CORE, idx))

    all_ks = sorted({k for g in per_core for k in g.keys()})
    raw_plan = []  # (k, c)
    for k in all_ks:
        n_max = max(len(g[k][0]) if k in g else 0 for g in per_core)
        n_pad = -(-n_max // 128) * 128
        while n_pad > 0:
            c = min(n_pad, CHUNK)
            if c == 384:
                raw_plan.append((k, 256))
                n_pad -= 256
                continue
            raw_plan.append((k, c))
            n_pad -= c
    # width-sorted: equal-width items adjacent so mm1 pairs/quads pack
    # fully; within the small tail width classes, deepest chains first
    chunk_plan = sorted(
        raw_plan,
        key=lambda kc: (-kc[1], kc[0] if kc[1] == 512 else -kc[0]))
    total_cells = sum(c for _, c in chunk_plan)
    items, pairs, quads = _plan_items(chunk_plan)
    vox_cols = sum(p[0][2] for p in pairs)

    vox_all = np.zeros((N_CORES, 128, vox_cols), ml_dtypes.bfloat16)
    rows_all = np.full((N_CORES, total_cells), PAD, np.int64)

    for core in range(N_CORES):
        groups = per_core[core]
        cell0 = 0
        used = {}
        src = {}
        for ci, (k, c) in enumerate(chunk_plan):
            cells, slots = groups.get(
                k, (np.zeros(0, np.int64), np.zeros((0, k), np.int64)))
            u = used.get(k, 0)
            batch_cells = cells[u : u + c]
            batch_slots = slots[u : u + c]
            used[k] = u + c
            nb = len(batch_cells)
            sl = np.zeros((c, k), np.int64)
            if nb:
                sl[:nb] = batch_slots
                sl[nb:] = batch_slots[0, 0]
            elif len(cells):
                sl[:] = slots[0, 0]
            for s in range(k):
                src[(ci, s)] = sl[:, s]
            rows_all[core, cell0 : cell0 + nb] = batch_cells
            cell0 += c
        col = 0
        vx = np.asarray(voxels, np.float32)
        for pr in pairs:
            c = pr[0][2]
            ia = src[(pr[0][0], pr[0][3])]
            vox_all[core, 0:4, col : col + c] = vx[ia].T
            if len(pr) == 2:
                ib = src[(pr[1][0], pr[1][3])]
                vox_all[core, 4:8, col : col + c] = vx[ib].T
            col += c
        assert col == vox_cols and cell0 == total_cells
    return chunk_plan, vox_cols, vox_all, rows_all


# ------------------------------------------------------------- bass program


def build_program(chunk_plan, vox_cols):
    fpairs, pair_of, pair_cells = _pair_chunks(chunk_plan)
    nc = bacc.Bacc("TRN2", target_bir_lowering=False, debug=False,
                   num_devices=N_CORES)

    vox = nc.dram_tensor("vox", [128, vox_cols], BF16,
                         kind="ExternalInput").ap()
    w_in = {}
    for name, shape, dt in [
        ("wpack", [128, 768], BF16), ("bpack", [128, 5], F32),
    ]:
        w_in[name] = nc.dram_tensor(name, shape, dt, kind="ExternalInput").ap()
    comp = nc.dram_tensor("comp", [128, pair_cells], BF16,
                          kind="ExternalOutput").ap()

    from contextlib import ExitStack
    with tile.TileContext(nc) as tc, ExitStack() as ctx:
        cpool = ctx.enter_context(tc.tile_pool(name="const", bufs=1))

        wpk = cpool.tile([128, 768], BF16, tag="wpack")
        nc.sync.dma_start(out=wpk[:], in_=w_in["wpack"])
        w1d = wpk[:, 0:128]
        w2e = wpk[:, 128:256]
        w2o = wpk[:, 256:384]
        w3a = wpk[:, 384:512]
        w3b = wpk[:, 512:640]
        wc0 = wpk[:, 640:704]
        wc1 = wpk[:, 704:768]
        bpk = cpool.tile([128, 5], F32, tag="bpack")
        nc.sync.dma_start(out=bpk[:], in_=w_in["bpack"])
        t1x = bpk[:, 0:1]
        t2 = bpk[:, 1:2]
        t3a = bpk[:, 2:3]
        t3b = bpk[:, 3:4]
        bcr = bpk[:, 4:5]

        sb = ctx.enter_context(tc.tile_pool(name="sb", bufs=8))
        scp = ctx.enter_context(tc.tile_pool(name="scp", bufs=3))
        vxp = ctx.enter_context(tc.tile_pool(name="vx", bufs=4))
        # PSUM (8 banks): p1 [128,<=512] x1, p2 [128,<=1024] x1,
        # psA/psB [128,<=1024] x1 each, pc [128,<=512] x1
        p1p = ctx.enter_context(tc.tile_pool(name="p1p", bufs=1, space="PSUM"))
        p2p = ctx.enter_context(tc.tile_pool(name="p2p", bufs=1, space="PSUM"))
        psA = ctx.enter_context(tc.tile_pool(name="psA", bufs=1, space="PSUM"))
        psB = ctx.enter_context(tc.tile_pool(name="psB", bufs=1, space="PSUM"))
        pcp = ctx.enter_context(tc.tile_pool(name="pcp", bufs=1, space="PSUM"))

        def ldw(w):
            pass

        def br_dve(out_ap, in_ap, bias_ap):
            nc.vector.tensor_scalar(
                out_ap, in_ap, bias_ap, 0.0,
                op0=mybir.AluOpType.add, op1=mybir.AluOpType.max)

        debt = {"act": 0.0, "dve": 0.0}

        def br_auto(out_ap, in_ap, bias_ap):
            c = in_ap.shape[-1]
            ca, cd = (c + 190) / 1.2, (c + 120) / 0.96
            if debt["act"] + ca <= debt["dve"] + cd:
                debt["act"] += ca
                nc.scalar.activation(out_ap, in_ap, Relu, bias=bias_ap,
                                     scale=1.0)
            else:
                debt["dve"] += cd
                br_dve(out_ap, in_ap, bias_ap)

        def dve_forced(c):
            debt["dve"] += (c + 120) / 0.96

        def br_split(out_ap, in_ap, bias_ap):
            # wide evac split across both engines: halves the PSUM
            # recycle latency at the cost of one extra op's overhead
            W = in_ap.shape[-1]
            h = W // 2
            debt["act"] += (h + 190) / 1.2
            nc.scalar.activation(out_ap[:, 0:h], in_ap[:, 0:h], Relu,
                                 bias=bias_ap, scale=1.0)
            debt["dve"] += (h + 120) / 0.96
            br_dve(out_ap[:, h:W], in_ap[:, h:W], bias_ap)

        def max_br_dve(out_ap, in_ap, bias_ap, acc_ap):
            nc.vector.scalar_tensor_tensor(
                out_ap, in_ap, bias_ap, acc_ap,
                op0=mybir.AluOpType.add, op1=mybir.AluOpType.max)

        # --- engine warmup: runs during the initial DMA wait ---
        # dummy ACT op forces the lazy ACT_TABLE_LOAD early; dummy matmul
        # chain keeps the PE busy so the HAM clock gate opens (2.4 GHz)
        # before the first real matmul; none of these touch DMA'd data
        dmy = cpool.tile([128, 128], BF16, tag="dmy")
        nc.vector.memset(dmy[:], 0.25)
        dbz = cpool.tile([128, 1], F32, tag="dbz")
        nc.vector.memset(dbz[:], 0.0)
        da = cpool.tile([128, 64], BF16, tag="da")
        nc.scalar.activation(da[:], dmy[:, 0:64], Relu, bias=dbz[:],
                             scale=1.0)
        dv = cpool.tile([128, 64], BF16, tag="dv")
        nc.vector.tensor_scalar(dv[:], dmy[:, 0:64], dbz[:], 0.0,
                                op0=mybir.AluOpType.add,
                                op1=mybir.AluOpType.max)
        pw = pcp.tile([128, 128], F32, tag="pc", space="PSUM",
                      name="warm")
        for wn in range(28):
            nc.tensor.matmul(pw[:], dmy[:], dmy[:], start=(wn == 0),
                             stop=(wn == 27))

        items, pairs, quads = _plan_items(chunk_plan)
        cell_off = []
        co = 0
        for ci, (k, c) in enumerate(chunk_plan):
            cell_off.append(co)
            co += c
        pair_off = {}
        po = 0
        for pi, pr in enumerate(pairs):
            pair_off[pi] = po
            po += pr[0][2]
        pair_idx = {id(pr): pi for pi, pr in enumerate(pairs)}

        # batched vox loads; first slab small so mm1 starts early
        vx_ap = {}
        batch = []
        bcols = 0
        slab_cap = [1024]

        def flush_vox():
            nonlocal batch, bcols
            if not batch:
                return
            p0 = pair_off[batch[0]]
            vx = vxp.tile([128, bcols], BF16, tag="vx", name=f"vx{p0}")
            nc.sync.dma_start(out=vx[:], in_=vox[:, p0 : p0 + bcols])
            for pi in batch:
                rel = pair_off[pi] - p0
                vx_ap[pi] = vx[:, rel : rel + pairs[pi][0][2]]
            batch = []
            bcols = 0
            slab_cap[0] = 4096

        for pi, pr in enumerate(pairs):
            if bcols + pr[0][2] > slab_cap[0]:
                flush_vox()
            batch.append(pi)
            bcols += pr[0][2]
        flush_vox()

        # chunk state
        accA = {}
        accB = {}
        uaccA = {}
        uaccB = {}
        ucnt = {}
        done_chunks = set()
        fin_q = []

        def chunk_done(ci, k, c):
            done_chunks.add(ci)
            fpi = pair_of[ci]
            ca, cb, off, cw = fpairs[fpi]
            if cb is None or (ca in done_chunks and cb in done_chunks):
                fin_q.append((fpi,))

        def finalize_pair(fpi):
            ca, cb, off, c = fpairs[fpi]
            pc = pcp.tile([128, c], F32, tag="pc", space="PSUM",
                          name=f"pc{fpi}")
            nc.tensor.matmul(pc[0:64, :], wc0, accA.pop(ca),
                             start=True, stop=False)
            if cb is not None:
                nc.tensor.matmul(pc[64:128, :], wc0, accA.pop(cb),
                                 start=True, stop=False)
            nc.tensor.matmul(pc[0:64, :], wc1, accB.pop(ca),
                             start=False, stop=True)
            if cb is not None:
                nc.tensor.matmul(pc[64:128, :], wc1, accB.pop(cb),
                                 start=False, stop=True)
            sc = scp.tile([128, c], BF16, tag="sc", name=f"sc{fpi}")
            br_auto(sc[:], pc[:], bcr)
            q = nc.sync if fpi % 2 else nc.gpsimd
            q.dma_start(out=comp[:, off : off + c], in_=sc[:])

        def stage1(quad):
            """block-diag mm1 per pair -> one wide p1 -> h1w (bf16)."""
            W = sum(pr[0][2] for pr in quad)
            nm = f"{quad[0][0][0]}_{quad[0][0][3]}"
            p1 = p1p.tile([128, W], F32, tag="p1", space="PSUM",
                          name=f"p1_{nm}")
            ldw(w1d)
            o = 0
            for pr in quad:
                c = pr[0][2]
                nc.tensor.matmul(p1[:, o : o + c], w1d,
                                 vx_ap[pair_idx[id(pr)]],
                                 start=True, stop=True)
                o += c
            h1 = sb.tile([128, W], BF16, tag="h1", name=f"h1_{nm}")
            br_auto(h1[:], p1[:], t1x)
            return h1

        def stage2(quad, h1):
            """mm2 all-evens then all-odds -> one wide p2 -> h2w (bf16)."""
            W = sum(len(pr) * pr[0][2] for pr in quad)
            nm = f"{quad[0][0][0]}_{quad[0][0][3]}"
            p2 = p2p.tile([128, W], F32, tag="p2", space="PSUM",
                          name=f"p2_{nm}")
            parts = []
            ne = sum(pr[0][2] for pr in quad)
            oe, oo, ho = 0, ne, 0
            ldw(w2e)
            for pr in quad:
                c = pr[0][2]
                nc.tensor.matmul(p2[:, oe : oe + c], w2e,
                                 h1[:, ho : ho + c], start=True, stop=True)
                parts.append((pr[0], oe))
                oe += c
                ho += c
            ldw(w2o)
            ho = 0
            for pr in quad:
                c = pr[0][2]
                if len(pr) == 2:
                    nc.tensor.matmul(p2[:, oo : oo + c], w2o,
                                     h1[:, ho : ho + c],
                                     start=True, stop=True)
                    parts.append((pr[1], oo))
                    oo += c
                ho += c
            h2 = sb.tile([128, W], BF16, tag="h2", name=f"h2_{nm}")
            br_auto(h2[:], p2[:], t2)
            # restore stream order for stage3 (chunk slots in order)
            parts.sort(key=lambda po_: po_[0][4])
            return h2, parts

        def stage3_duo(todo):
            """mm3a/b for <=2 items sharing wide psA/psB tiles."""
            W = sum(it[2] for it, _ in todo)
            nm = f"{todo[0][0][0]}_{todo[0][0][3]}"
            pA = psA.tile([128, W], F32, tag="pA", space="PSUM",
                          name=f"pA_{nm}")
            pB = psB.tile([128, W], F32, tag="pB", space="PSUM",
                          name=f"pB_{nm}")
            ldw(w3a)
            o = 0
            for it, h2_ap in todo:
                nc.tensor.matmul(pA[:, o : o + it[2]], w3a, h2_ap,
                                 start=True, stop=True)
                o += it[2]
            ldw(w3b)
            o = 0
            for it, h2_ap in todo:
                nc.tensor.matmul(pB[:, o : o + it[2]], w3b, h2_ap,
                                 start=True, stop=True)
                o += it[2]
            # evacuation
            if (len(todo) == 2 and todo[0][0][1] == 1 and todo[1][0][1] == 1):
                # two single-slot chunks: one wide bias+relu per half
                c0 = todo[0][0][2]
                ciX, ciY = todo[0][0][0], todo[1][0][0]
                aA = sb.tile([128, W], BF16, tag="accrA", name=f"aA_{ciX}w")
                aB = sb.tile([128, W], BF16, tag="accrB", name=f"aB_{ciX}w")
                br_auto(aA[:], pA[:], t3a)
                br_auto(aB[:], pB[:], t3b)
                accA[ciX], accB[ciX] = aA[:, 0:c0], aB[:, 0:c0]
                accA[ciY], accB[ciY] = aA[:, c0:W], aB[:, c0:W]
                for it, _ in todo:
                    chunk_done(it[0], it[1], it[2])
            else:
                o = 0
                for it, _ in todo:
                    slot_ops(it, pA[:, o : o + it[2]], pB[:, o : o + it[2]])
                    o += it[2]

        def slot_ops(it, pA_ap, pB_ap):
            ci, k, c, s, po_ = it
            if k == 1:
                aA = sb.tile([128, c], BF16, tag="accrA", name=f"aA_{ci}")
                br_auto(aA[:], pA_ap, t3a)
                aB = sb.tile([128, c], BF16, tag="accrB", name=f"aB_{ci}")
                br_auto(aB[:], pB_ap, t3b)
                accA[ci], accB[ci] = aA[:], aB[:]
            elif s == 0:
                aA = [sb.tile([128, c], BF16, tag="accrA",
                              name=f"aA_{ci}_{j}") for j in range(2)]
                aB = [sb.tile([128, c], BF16, tag="accrB",
                              name=f"aB_{ci}_{j}") for j in range(2)]
                br_auto(aA[0][:], pA_ap, t3a)
                br_auto(aB[0][:], pB_ap, t3b)
                accA[ci] = aA
                accB[ci] = aB
                accA[f"n{ci}"] = 0
                accB[f"n{ci}"] = 0
            else:
                nA = accA[f"n{ci}"]
                max_br_dve(accA[ci][1 - nA][:], pA_ap, t3a,
                           accA[ci][nA][:])
                accA[f"n{ci}"] = 1 - nA
                nB = accB[f"n{ci}"]
                max_br_dve(accB[ci][1 - nB][:], pB_ap, t3b,
                           accB[ci][nB][:])
                accB[f"n{ci}"] = 1 - nB
                dve_forced(c)
                dve_forced(c)
            if s == k - 1:
                if k > 1:
                    nA, nB = accA.pop(f"n{ci}"), accB.pop(f"n{ci}")
                    accA[ci] = accA[ci][nA][:]
                    accB[ci] = accB[ci][nB][:]
                chunk_done(ci, k, c)

        # software-pipelined emission over quads
        h1q = []
        h2q = []
        n_quads = len(quads)
        for t in range(n_quads + 3):
            ready_fins = list(fin_q)
            fin_q.clear()
            if t < n_quads:
                h1q.append((quads[t], stage1(quads[t])))
            if t >= 1 and h1q:
                qd, h1 = h1q.pop(0)
                h2q.append(stage2(qd, h1))
            if t >= 2 and h2q:
                h2, parts = h2q.pop(0)
                j = 0
                while j < len(parts):
                    duo = [parts[j]]
                    it0 = parts[j][0]
                    take2 = False
                    if j + 1 < len(parts):
                        it1 = parts[j + 1][0]
                        if it0[1] == 1 and it1[1] == 1:
                            take2 = (it0[2] == 512
                                     or it0[2] + it1[2] <= 512)
                        elif it0[0] == it1[0]:
                            take2 = True
                    if take2:
                        duo.append(parts[j + 1])
                        j += 2
                    else:
                        j += 1
                    stage3_duo([(it, h2[:, o : o + it[2]]) for it, o in duo])
            for f in ready_fins:
                finalize_pair(*f)

    nc.compile()
    return nc


# ------------------------------------------------------------------ driver

_CACHE = {}


def kernel(voxels, coors, batch_size, w1, g1, b1, m1, v1,
           w2, g2, b2, m2, v2, w3, g3, b3, m3, v3, wc, bc,
           _trace=False):
    voxels = np.asarray(voxels, np.float32)
    coors = np.asarray(coors, np.int32)
    chunk_plan, vox_cols, vox_all, rows_all = _build_plan_and_data(
        voxels, coors)

    key = tuple(chunk_plan)
    if key not in _CACHE:
        _CACHE[key] = build_program(chunk_plan, vox_cols)
    nc = _CACHE[key]

    def fold(w, g, b, m, v):
        s = np.asarray(g, np.float32) / np.sqrt(
            np.asarray(v, np.float32) + np.float32(EPS))
        wf = np.asarray(w, np.float32) * s[None, :]
        t = np.asarray(b, np.float32) - np.asarray(m, np.float32) * s
        return wf, t

    w1f, t1 = fold(w1, g1, b1, m1, v1)
    w2f, t2 = fold(w2, g2, b2, m2, v2)
    w3f, t3 = fold(w3, g3, b3, m3, v3)

    bf = ml_dtypes.bfloat16
    w1dm = np.zeros((128, 128), np.float32)
    w1dm[0:4, 0:64] = w1f
    w1dm[4:8, 64:128] = w1f
    w2em = np.zeros((128, 128), np.float32)
    w2em[0:64, :] = w2f
    w2om = np.zeros((128, 128), np.float32)
    w2om[64:128, :] = w2f
    wcf = np.asarray(wc, np.float32)
    wc0p = np.zeros((128, 128), np.float32)
    wc0p[:, 0:64] = wcf[0:128]
    wc1p = np.zeros((128, 128), np.float32)
    wc1p[:, 0:64] = wcf[128:256]
    wc0q = np.zeros((128, 128), np.float32)
    wc0q[:, 64:128] = wcf[0:128]
    wc1q = np.zeros((128, 128), np.float32)
    wc1q[:, 64:128] = wcf[128:256]
    t1xm = np.concatenate([t1, t1]).astype(np.float32)
    bcf_ = np.asarray(bc, np.float32)
    bcu_ = bcf_ + t3.astype(np.float32) @ wcf
    bcrm = np.concatenate([bcf_, bcf_]).astype(np.float32)
    bcum = np.concatenate([bcu_, bcu_]).astype(np.float32)

    wpack = np.zeros((128, 768), np.float32)
    wpack[:, 0:128] = w1dm
    wpack[:, 128:256] = w2em
    wpack[:, 256:384] = w2om
    wpack[:, 384:512] = np.ascontiguousarray(w3f[:, 0:128])
    wpack[:, 512:640] = np.ascontiguousarray(w3f[:, 128:256])
    wpack[:, 640:704] = wcf[0:128]
    wpack[:, 704:768] = wcf[128:256]
    bpack = np.zeros((128, 5), np.float32)
    bpack[:, 0] = t1xm
    bpack[:, 1] = t2
    bpack[:, 2] = t3[0:128]
    bpack[:, 3] = t3[128:256]
    bpack[:, 4] = bcrm
    weights = {
        "wpack": wpack.astype(bf),
        "bpack": bpack,
    }
    in_maps = [{"vox": vox_all[c], **weights} for c in range(N_CORES)]
    res = bass_utils.run_bass_kernel_spmd(
        nc, in_maps, core_ids=list(range(N_CORES)), trace=_trace)

    fpairs, pair_of, pair_cells = _pair_chunks(chunk_plan)
    cell_off = []
    co = 0
    for ci, (k, c) in enumerate(chunk_plan):
        cell_off.append(co)
        co += c
    total_cells = co

    out = np.zeros((B, 64, GX * GY), np.float32)
    for c in range(N_CORES):
        cp = res.results[c]["comp"].astype(np.float32)  # [128, pair_cells]
        cm = np.zeros((64, total_cells), np.float32)
        for ca, cb, off, cw in fpairs:
            cm[:, cell_off[ca] : cell_off[ca] + cw] = cp[0:64, off : off + cw]
            if cb is not None:
                cm[:, cell_off[cb] : cell_off[cb] + cw] = (
                    cp[64:128, off : off + cw])
        rows = rows_all[c]
        real = rows != PAD
        gcell = rows[real] + c * CELLS_PER_CORE
        b_core = c // (N_CORES // B)
        xy = gcell - b_core * (GX * GY)
        out[b_core][:, xy] = cm[:, real]
    out = out.reshape(B, 64, GX, GY)
    if _trace:
        return out, res
    return out


# revision 50
# speedup vs baseline: 1.0088x; 1.0069x over previous
"""BevFeatureEncoder on 8 Trainium2 NeuronCores.

Strategy (data-parallel over BEV grid slabs):
  - The 2*480*360 BEV cells are split into 8 contiguous ranges of 43200
    cells. Points are routed on host to the core owning their cell, so
    the segment_max reduction is fully local to each core.
  - On host (integer indexing only), each core's occupied cells are
    grouped by point count, counts padded up to k by duplicating points
    of the same cell (a no-op under max). Cells are processed in chunks
    of <=512; points are laid out so slot s of a chunk is a dense,
    contiguous block of points.
  - BN scale/bias are folded into the weights ON HOST; all matmuls are
    K=128, M=128, bf16 (uniform PE configuration: no weight-path
    stalls from contraction-depth or dtype switches; FWL fast loads).
  - Layer 1 uses a block-diagonal weight (two w1 copies): each moving
    column carries TWO points, halving mm1 columns.
  - Dummy warmup ops run during the initial DMA wait: an early ACT op
    forces the lazy ACT_TABLE_LOAD, and a 28-matmul PSUM accumulation
    group opens the HAM clock gate before the first real matmul.
  - Two chunks are compressed into one [128, c] PSUM tile (second wc
    copy shifted to output partitions 64-127), halving the final
    bias+relu ops and giving dense output DMA.
"""

import numpy as np
import ml_dtypes

import concourse.bacc as bacc
import concourse.bass as bass
import concourse.mybir as mybir
import concourse.tile as tile
from concourse import bass_utils

GX, GY = 480, 360
B = 2
EPS = 1e-5
N_CORES = 8
CELLS_PER_CORE = (B * GX * GY) // N_CORES  # 43200
CHUNK = 512
PAD = -1

F32 = mybir.dt.float32
BF16 = mybir.dt.bfloat16

Relu = mybir.ActivationFunctionType.Relu


# ---------------------------------------------------------------- host prep


def _prep_core(seg_local, lo_idx):
    """Group one core's occupied cells by padded point count."""
    order = np.argsort(seg_local, kind="stable")
    seg_sorted = seg_local[order]
    cells, starts, counts = np.unique(
        seg_sorted, return_index=True, return_counts=True
    )
    ks2 = 1 << (np.ceil(np.log2(np.maximum(counts, 1))).astype(np.int64))
    ks = np.where(counts <= 4, counts, np.maximum(ks2, 1)).astype(np.int64)
    out = {}
    for k in np.unique(ks):
        sel = np.nonzero(ks == k)[0]
        slots = np.empty((len(sel), int(k)), np.int64)
        for s in range(int(k)):
            slots[:, s] = order[starts[sel] + np.minimum(s, counts[sel] - 1)]
        out[int(k)] = (cells[sel].astype(np.int64), lo_idx[slots])
    return out


def _plan_items(chunk_plan):
    """Flat slot-item stream (ci, k, c, s, pt_off) and mm1 item pairs."""
    items = []
    pt = 0
    for ci, (k, c) in enumerate(chunk_plan):
        for s in range(k):
            items.append((ci, k, c, s, pt))
            pt += c
    # mm1 pairs: consecutive equal-width items share one block-diag matmul
    pairs = []
    i = 0
    while i < len(items):
        if i + 1 < len(items) and items[i + 1][2] == items[i][2]:
            pairs.append([items[i], items[i + 1]])
            i += 2
        else:
            pairs.append([items[i]])
            i += 1
    # quads: consecutive pairs share wide PSUM tiles; stage-2 tile (one
    # column per ITEM) <= 1024, stage-1 tile (one column per PAIR) <= 512
    quads = []
    i = 0
    while i < len(pairs):
        quad = [pairs[i]]
        w1 = pairs[i][0][2]
        w2 = len(pairs[i]) * pairs[i][0][2]
        i += 1
        while (i < len(pairs)
               and w2 + len(pairs[i]) * pairs[i][0][2] <= 1024
               and w1 + pairs[i][0][2] <= 512):
            quad.append(pairs[i])
            w1 += pairs[i][0][2]
            w2 += len(pairs[i]) * pairs[i][0][2]
            i += 1
        quads.append(quad)
    return items, pairs, quads


def _pair_chunks(chunk_plan):
    """Pair chunks (in completion order) for the 2-in-1 compression tile.

    Returns (fpairs, pair_of, pair_cells): fpairs is a list of
    (ci, cj_or_None, out_off, c); pair_of maps ci -> fpair index."""
    fpairs = []
    pair_of = {}
    off = 0
    pend = None  # (ci, c, k)
    for ci, (k, c) in enumerate(chunk_plan):
        if pend is not None and pend[1] == c:
            fp = (pend[0], ci, off, c)
            pair_of[pend[0]] = len(fpairs)
            pair_of[ci] = len(fpairs)
            fpairs.append(fp)
            off += c
            pend = None
        else:
            if pend is not None:
                fp = (pend[0], None, off, pend[1])
                pair_of[pend[0]] = len(fpairs)
                fpairs.append(fp)
                off += pend[1]
            pend = (ci, c, k)
    if pend is not None:
        fp = (pend[0], None, off, pend[1])
        pair_of[pend[0]] = len(fpairs)
        fpairs.append(fp)
        off += pend[1]
    return fpairs, pair_of, off


def _build_plan_and_data(voxels, coors):
    """Route points to cores, build the equalized chunk plan plus per-core
    device inputs (block-diag packed voxels, bf16) and placement tables."""
    seg = (
        coors[:, 0].astype(np.int64) * (GX * GY)
        + coors[:, 1].astype(np.int64) * GY
        + coors[:, 2].astype(np.int64)
    )
    core_of = seg // CELLS_PER_CORE
    per_core = []
    for c in range(N_CORES):
        idx = np.nonzero(core_of == c)[0]
        per_core.append(_prep_core(seg[idx] - c * CELLS_PER_CORE, idx))

    all_ks = sorted({k for g in per_core for k in g.keys()})
    raw_plan = []  # (k, c)
    for k in all_ks:
        n_max = max(len(g[k][0]) if k in g else 0 for g in per_core)
        n_pad = -(-n_max // 128) * 128
        while n_pad > 0:
            c = min(n_pad, CHUNK)
            if c == 384:
                raw_plan.append((k, 256))
                n_pad -= 256
                continue
            raw_plan.append((k, c))
            n_pad -= c
    # width-sorted: equal-width items adjacent so mm1 pairs/quads pack
    # fully; within the small tail width classes, deepest chains first
    chunk_plan = sorted(
        raw_plan,
        key=lambda kc: (-kc[1], kc[0] if kc[1] == 512 else -kc[0]))
    total_cells = sum(c for _, c in chunk_plan)
    items, pairs, quads = _plan_items(chunk_plan)
    vox_cols = sum(p[0][2] for p in pairs)

    vox_all = np.zeros((N_CORES, 128, vox_cols), ml_dtypes.bfloat16)
    rows_all = np.full((N_CORES, total_cells), PAD, np.int64)

    for core in range(N_CORES):
        groups = per_core[core]
        cell0 = 0
        used = {}
        src = {}
        for ci, (k, c) in enumerate(chunk_plan):
            cells, slots = groups.get(
                k, (np.zeros(0, np.int64), np.zeros((0, k), np.int64)))
            u = used.get(k, 0)
            batch_cells = cells[u : u + c]
            batch_slots = slots[u : u + c]
            used[k] = u + c
            nb = len(batch_cells)
            sl = np.zeros((c, k), np.int64)
            if nb:
                sl[:nb] = batch_slots
                sl[nb:] = batch_slots[0, 0]
            elif len(cells):
                sl[:] = slots[0, 0]
            for s in range(k):
                src[(ci, s)] = sl[:, s]
            rows_all[core, cell0 : cell0 + nb] = batch_cells
            cell0 += c
        col = 0
        vx = np.asarray(voxels, np.float32)
        for pr in pairs:
            c = pr[0][2]
            ia = src[(pr[0][0], pr[0][3])]
            vox_all[core, 0:4, col : col + c] = vx[ia].T
            if len(pr) == 2:
                ib = src[(pr[1][0], pr[1][3])]
                vox_all[core, 4:8, col : col + c] = vx[ib].T
            col += c
        assert col == vox_cols and cell0 == total_cells
    return chunk_plan, vox_cols, vox_all, rows_all


# ------------------------------------------------------------- bass program


def build_program(chunk_plan, vox_cols):
    fpairs, pair_of, pair_cells = _pair_chunks(chunk_plan)
    nc = bacc.Bacc("TRN2", target_bir_lowering=False, debug=False,
                   num_devices=N_CORES)

    vox = nc.dram_tensor("vox", [128, vox_cols], BF16,
                         kind="ExternalInput").ap()
    w_in = {}
    for name, shape, dt in [
        ("wpack", [128, 768], BF16), ("bpack", [128, 5], F32),
    ]:
        w_in[name] = nc.dram_tensor(name, shape, dt, kind="ExternalInput").ap()
    comp = nc.dram_tensor("comp", [128, pair_cells], BF16,
                          kind="ExternalOutput").ap()

    from contextlib import ExitStack
    with tile.TileContext(nc) as tc, ExitStack() as ctx:
        cpool = ctx.enter_context(tc.tile_pool(name="const", bufs=1))

        wpk = cpool.tile([128, 768], BF16, tag="wpack")
        nc.sync.dma_start(out=wpk[:], in_=w_in["wpack"])
        w1d = wpk[:, 0:128]
        w2e = wpk[:, 128:256]
        w2o = wpk[:, 256:384]
        w3a = wpk[:, 384:512]
        w3b = wpk[:, 512:640]
        wc0 = wpk[:, 640:704]
        wc1 = wpk[:, 704:768]
        bpk = cpool.tile([128, 5], F32, tag="bpack")
        nc.sync.dma_start(out=bpk[:], in_=w_in["bpack"])
        t1x = bpk[:, 0:1]
        t2 = bpk[:, 1:2]
        t3a = bpk[:, 2:3]
        t3b = bpk[:, 3:4]
        bcr = bpk[:, 4:5]

        sb = ctx.enter_context(tc.tile_pool(name="sb", bufs=8))
        scp = ctx.enter_context(tc.tile_pool(name="scp", bufs=3))
        vxp = ctx.enter_context(tc.tile_pool(name="vx", bufs=4))
        # PSUM (8 banks): p1 [128,<=512] x1, p2 [128,<=1024] x1,
        # psA/psB [128,<=1024] x1 each, pc [128,<=512] x1
        p1p = ctx.enter_context(tc.tile_pool(name="p1p", bufs=1, space="PSUM"))
        p2p = ctx.enter_context(tc.tile_pool(name="p2p", bufs=1, space="PSUM"))
        psA = ctx.enter_context(tc.tile_pool(name="psA", bufs=1, space="PSUM"))
        psB = ctx.enter_context(tc.tile_pool(name="psB", bufs=1, space="PSUM"))
        pcp = ctx.enter_context(tc.tile_pool(name="pcp", bufs=1, space="PSUM"))

        def ldw(w):
            pass

        def br_dve(out_ap, in_ap, bias_ap):
            nc.vector.tensor_scalar(
                out_ap, in_ap, bias_ap, 0.0,
                op0=mybir.AluOpType.add, op1=mybir.AluOpType.max)

        debt = {"act": 0.0, "dve": 0.0}

        def br_auto(out_ap, in_ap, bias_ap):
            c = in_ap.shape[-1]
            ca, cd = (c + 190) / 1.2, (c + 120) / 0.90
            if debt["act"] + ca <= debt["dve"] + cd:
                debt["act"] += ca
                nc.scalar.activation(out_ap, in_ap, Relu, bias=bias_ap,
                                     scale=1.0)
            else:
                debt["dve"] += cd
                br_dve(out_ap, in_ap, bias_ap)

        def dve_forced(c):
            debt["dve"] += (c + 120) / 0.90

        def br_split(out_ap, in_ap, bias_ap):
            # wide evac split across both engines: halves the PSUM
            # recycle latency at the cost of one extra op's overhead
            W = in_ap.shape[-1]
            h = W // 2
            debt["act"] += (h + 190) / 1.2
            nc.scalar.activation(out_ap[:, 0:h], in_ap[:, 0:h], Relu,
                                 bias=bias_ap, scale=1.0)
            debt["dve"] += (h + 120) / 0.96
            br_dve(out_ap[:, h:W], in_ap[:, h:W], bias_ap)

        def max_br_dve(out_ap, in_ap, bias_ap, acc_ap):
            nc.vector.scalar_tensor_tensor(
                out_ap, in_ap, bias_ap, acc_ap,
                op0=mybir.AluOpType.add, op1=mybir.AluOpType.max)

        # --- engine warmup: runs during the initial DMA wait ---
        # dummy ACT op forces the lazy ACT_TABLE_LOAD early; dummy matmul
        # chain keeps the PE busy so the HAM clock gate opens (2.4 GHz)
        # before the first real matmul; none of these touch DMA'd data
        dmy = cpool.tile([128, 128], BF16, tag="dmy")
        nc.vector.memset(dmy[:], 0.25)
        dbz = cpool.tile([128, 1], F32, tag="dbz")
        nc.vector.memset(dbz[:], 0.0)
        da = cpool.tile([128, 64], BF16, tag="da")
        nc.scalar.activation(da[:], dmy[:, 0:64], Relu, bias=dbz[:],
                             scale=1.0)
        dv = cpool.tile([128, 64], BF16, tag="dv")
        nc.vector.tensor_scalar(dv[:], dmy[:, 0:64], dbz[:], 0.0,
                                op0=mybir.AluOpType.add,
                                op1=mybir.AluOpType.max)
        pw = pcp.tile([128, 128], F32, tag="pc", space="PSUM",
                      name="warm")
        for wn in range(28):
            nc.tensor.matmul(pw[:], dmy[:], dmy[:], start=(wn == 0),
                             stop=(wn == 27))

        items, pairs, quads = _plan_items(chunk_plan)
        cell_off = []
        co = 0
        for ci, (k, c) in enumerate(chunk_plan):
            cell_off.append(co)
            co += c
        pair_off = {}
        po = 0
        for pi, pr in enumerate(pairs):
            pair_off[pi] = po
            po += pr[0][2]
        pair_idx = {id(pr): pi for pi, pr in enumerate(pairs)}

        # batched vox loads; first slab small so mm1 starts early
        vx_ap = {}
        batch = []
        bcols = 0
        slab_cap = [1024]

        def flush_vox():
            nonlocal batch, bcols
            if not batch:
                return
            p0 = pair_off[batch[0]]
            vx = vxp.tile([128, bcols], BF16, tag="vx", name=f"vx{p0}")
            nc.sync.dma_start(out=vx[:], in_=vox[:, p0 : p0 + bcols])
            for pi in batch:
                rel = pair_off[pi] - p0
                vx_ap[pi] = vx[:, rel : rel + pairs[pi][0][2]]
            batch = []
            bcols = 0
            slab_cap[0] = 4096

        for pi, pr in enumerate(pairs):
            if bcols + pr[0][2] > slab_cap[0]:
                flush_vox()
            batch.append(pi)
            bcols += pr[0][2]
        flush_vox()

        # chunk state
        accA = {}
        accB = {}
        uaccA = {}
        uaccB = {}
        ucnt = {}
        done_chunks = set()
        fin_q = []

        def chunk_done(ci, k, c):
            done_chunks.add(ci)
            fpi = pair_of[ci]
            ca, cb, off, cw = fpairs[fpi]
            if cb is None or (ca in done_chunks and cb in done_chunks):
                fin_q.append((fpi,))

        def finalize_pair(fpi):
            ca, cb, off, c = fpairs[fpi]
            pc = pcp.tile([128, c], F32, tag="pc", space="PSUM",
                          name=f"pc{fpi}")
            nc.tensor.matmul(pc[0:64, :], wc0, accA.pop(ca),
                             start=True, stop=False)
            if cb is not None:
                nc.tensor.matmul(pc[64:128, :], wc0, accA.pop(cb),
                                 start=True, stop=False)
            nc.tensor.matmul(pc[0:64, :], wc1, accB.pop(ca),
                             start=False, stop=True)
            if cb is not None:
                nc.tensor.matmul(pc[64:128, :], wc1, accB.pop(cb),
                                 start=False, stop=True)
            sc = scp.tile([128, c], BF16, tag="sc", name=f"sc{fpi}")
            br_auto(sc[:], pc[:], bcr)
            q = nc.sync if fpi % 2 else nc.gpsimd
            q.dma_start(out=comp[:, off : off + c], in_=sc[:])

        def stage1(quad):
            """block-diag mm1 per pair -> one wide p1 -> h1w (bf16)."""
            W = sum(pr[0][2] for pr in quad)
            nm = f"{quad[0][0][0]}_{quad[0][0][3]}"
            p1 = p1p.tile([128, W], F32, tag="p1", space="PSUM",
                          name=f"p1_{nm}")
            ldw(w1d)
            o = 0
            for pr in quad:
                c = pr[0][2]
                nc.tensor.matmul(p1[:, o : o + c], w1d,
                                 vx_ap[pair_idx[id(pr)]],
                                 start=True, stop=True)
                o += c
            h1 = sb.tile([128, W], BF16, tag="h1", name=f"h1_{nm}")
            br_auto(h1[:], p1[:], t1x)
            return h1

        def stage2(quad, h1):
            """mm2 all-evens then all-odds -> one wide p2 -> h2w (bf16)."""
            W = sum(len(pr) * pr[0][2] for pr in quad)
            nm = f"{quad[0][0][0]}_{quad[0][0][3]}"
            p2 = p2p.tile([128, W], F32, tag="p2", space="PSUM",
                          name=f"p2_{nm}")
            parts = []
            ne = sum(pr[0][2] for pr in quad)
            oe, oo, ho = 0, ne, 0
            ldw(w2e)
            for pr in quad:
                c = pr[0][2]
                nc.tensor.matmul(p2[:, oe : oe + c], w2e,
                                 h1[:, ho : ho + c], start=True, stop=True)
                parts.append((pr[0], oe))
                oe += c
                ho += c
            ldw(w2o)
            ho = 0
            for pr in quad:
                c = pr[0][2]
                if len(pr) == 2:
                    nc.tensor.matmul(p2[:, oo : oo + c], w2o,
                                     h1[:, ho : ho + c],
                                     start=True, stop=True)
                    parts.append((pr[1], oo))
                    oo += c
                ho += c
            h2 = sb.tile([128, W], BF16, tag="h2", name=f"h2_{nm}")
            br_auto(h2[:], p2[:], t2)
            # restore stream order for stage3 (chunk slots in order)
            parts.sort(key=lambda po_: po_[0][4])
            return h2, parts

        def stage3_duo(todo):
            """mm3a/b for <=2 items sharing wide psA/psB tiles."""
            W = sum(it[2] for it, _ in todo)
            nm = f"{todo[0][0][0]}_{todo[0][0][3]}"
            pA = psA.tile([128, W], F32, tag="pA", space="PSUM",
                          name=f"pA_{nm}")
            pB = psB.tile([128, W], F32, tag="pB", space="PSUM",
                          name=f"pB_{nm}")
            ldw(w3a)
            o = 0
            for it, h2_ap in todo:
                nc.tensor.matmul(pA[:, o : o + it[2]], w3a, h2_ap,
                                 start=True, stop=True)
                o += it[2]
            ldw(w3b)
            o = 0
            for it, h2_ap in todo:
                nc.tensor.matmul(pB[:, o : o + it[2]], w3b, h2_ap,
                                 start=True, stop=True)
                o += it[2]
            # evacuation
            if (len(todo) == 2 and todo[0][0][1] == 1 and todo[1][0][1] == 1):
                # two single-slot chunks: one wide bias+relu per half
                c0 = todo[0][0][2]
                ciX, ciY = todo[0][0][0], todo[1][0][0]
                aA = sb.tile([128, W], BF16, tag="accrA", name=f"aA_{ciX}w")
                aB = sb.tile([128, W], BF16, tag="accrB", name=f"aB_{ciX}w")
                br_auto(aA[:], pA[:], t3a)
                br_auto(aB[:], pB[:], t3b)
                accA[ciX], accB[ciX] = aA[:, 0:c0], aB[:, 0:c0]
                accA[ciY], accB[ciY] = aA[:, c0:W], aB[:, c0:W]
                for it, _ in todo:
                    chunk_done(it[0], it[1], it[2])
            else:
                o = 0
                for it, _ in todo:
                    slot_ops(it, pA[:, o : o + it[2]], pB[:, o : o + it[2]])
                    o += it[2]

        def slot_ops(it, pA_ap, pB_ap):
            ci, k, c, s, po_ = it
            if k == 1:
                aA = sb.tile([128, c], BF16, tag="accrA", name=f"aA_{ci}")
                br_auto(aA[:], pA_ap, t3a)
                aB = sb.tile([128, c], BF16, tag="accrB", name=f"aB_{ci}")
                br_auto(aB[:], pB_ap, t3b)
                accA[ci], accB[ci] = aA[:], aB[:]
            elif s == 0:
                aA = [sb.tile([128, c], BF16, tag="accrA",
                              name=f"aA_{ci}_{j}") for j in range(2)]
                aB = [sb.tile([128, c], BF16, tag="accrB",
                              name=f"aB_{ci}_{j}") for j in range(2)]
                br_auto(aA[0][:], pA_ap, t3a)
                br_auto(aB[0][:], pB_ap, t3b)
                accA[ci] = aA
                accB[ci] = aB
                accA[f"n{ci}"] = 0
                accB[f"n{ci}"] = 0
            else:
                nA = accA[f"n{ci}"]
                max_br_dve(accA[ci][1 - nA][:], pA_ap, t3a,
                           accA[ci][nA][:])
                accA[f"n{ci}"] = 1 - nA
                nB = accB[f"n{ci}"]
                max_br_dve(accB[ci][1 - nB][:], pB_ap, t3b,
                           accB[ci][nB][:])
                accB[f"n{ci}"] = 1 - nB
                dve_forced(c)
                dve_forced(c)
            if s == k - 1:
                if k > 1:
                    nA, nB = accA.pop(f"n{ci}"), accB.pop(f"n{ci}")
                    accA[ci] = accA[ci][nA][:]
                    accB[ci] = accB[ci][nB][:]
                chunk_done(ci, k, c)

        # software-pipelined emission over quads
        h1q = []
        h2q = []
        n_quads = len(quads)
        for t in range(n_quads + 3):
            ready_fins = list(fin_q)
            fin_q.clear()
            if t < n_quads:
                h1q.append((quads[t], stage1(quads[t])))
            if t >= 1 and h1q:
                qd, h1 = h1q.pop(0)
                h2q.append(stage2(qd, h1))
            if t >= 2 and h2q:
                h2, parts = h2q.pop(0)
                j = 0
                while j < len(parts):
                    duo = [parts[j]]
                    it0 = parts[j][0]
                    take2 = False
                    if j + 1 < len(parts):
                        it1 = parts[j + 1][0]
                        if it0[1] == 1 and it1[1] == 1:
                            take2 = (it0[2] == 512
                                     or it0[2] + it1[2] <= 512)
                        elif it0[0] == it1[0]:
                            take2 = True
                    if take2:
                        duo.append(parts[j + 1])
                        j += 2
                    else:
                        j += 1
                    stage3_duo([(it, h2[:, o : o + it[2]]) for it, o in duo])
            for f in ready_fins:
                finalize_pair(*f)

    nc.compile()
    return nc


# ------------------------------------------------------------------ driver

_CACHE = {}


def kernel(voxels, coors, batch_size, w1, g1, b1, m1, v1,
           w2, g2, b2, m2, v2, w3, g3, b3, m3, v3, wc, bc,
           _trace=False):
    voxels = np.asarray(voxels, np.float32)
    coors = np.asarray(coors, np.int32)
    chunk_plan, vox_cols, vox_all, rows_all = _build_plan_and_data(
        voxels, coors)

    key = tuple(chunk_plan)
    if key not in _CACHE:
        _CACHE[key] = build_program(chunk_plan, vox_cols)
    nc = _CACHE[key]

    def fold(w, g, b, m, v):
        s = np.asarray(g, np.float32) / np.sqrt(
            np.asarray(v, np.float32) + np.float32(EPS))
        wf = np.asarray(w, np.float32) * s[None, :]
        t = np.asarray(b, np.float32) - np.asarray(m, np.float32) * s
        return wf, t

    w1f, t1 = fold(w1, g1, b1, m1, v1)
    w2f, t2 = fold(w2, g2, b2, m2, v2)
    w3f, t3 = fold(w3, g3, b3, m3, v3)

    bf = ml_dtypes.bfloat16
    w1dm = np.zeros((128, 128), np.float32)
    w1dm[0:4, 0:64] = w1f
    w1dm[4:8, 64:128] = w1f
    w2em = np.zeros((128, 128), np.float32)
    w2em[0:64, :] = w2f
    w2om = np.zeros((128, 128), np.float32)
    w2om[64:128, :] = w2f
    wcf = np.asarray(wc, np.float32)
    wc0p = np.zeros((128, 128), np.float32)
    wc0p[:, 0:64] = wcf[0:128]
    wc1p = np.zeros((128, 128), np.float32)
    wc1p[:, 0:64] = wcf[128:256]
    wc0q = np.zeros((128, 128), np.float32)
    wc0q[:, 64:128] = wcf[0:128]
    wc1q = np.zeros((128, 128), np.float32)
    wc1q[:, 64:128] = wcf[128:256]
    t1xm = np.concatenate([t1, t1]).astype(np.float32)
    bcf_ = np.asarray(bc, np.float32)
    bcu_ = bcf_ + t3.astype(np.float32) @ wcf
    bcrm = np.concatenate([bcf_, bcf_]).astype(np.float32)
    bcum = np.concatenate([bcu_, bcu_]).astype(np.float32)

    wpack = np.zeros((128, 768), np.float32)
    wpack[:, 0:128] = w1dm
    wpack[:, 128:256] = w2em
    wpack[:, 256:384] = w2om
    wpack[:, 384:512] = np.ascontiguousarray(w3f[:, 0:128])
    wpack[:, 512:640] = np.ascontiguousarray(w3f[:, 128:256])
    wpack[:, 640:704] = wcf[0:128]
    wpack[:, 704:768] = wcf[128:256]
    bpack = np.zeros((128, 5), np.float32)
    bpack[:, 0] = t1xm
    bpack[:, 1] = t2
    bpack[:, 2] = t3[0:128]
    bpack[:, 3] = t3[128:256]
    bpack[:, 4] = bcrm
    weights = {
        "wpack": wpack.astype(bf),
        "bpack": bpack,
    }
    in_maps = [{"vox": vox_all[c], **weights} for c in range(N_CORES)]
    res = bass_utils.run_bass_kernel_spmd(
        nc, in_maps, core_ids=list(range(N_CORES)), trace=_trace)

    fpairs, pair_of, pair_cells = _pair_chunks(chunk_plan)
    cell_off = []
    co = 0
    for ci, (k, c) in enumerate(chunk_plan):
        cell_off.append(co)
        co += c
    total_cells = co

    out = np.zeros((B, 64, GX * GY), np.float32)
    for c in range(N_CORES):
        cp = res.results[c]["comp"].astype(np.float32)  # [128, pair_cells]
        cm = np.zeros((64, total_cells), np.float32)
        for ca, cb, off, cw in fpairs:
            cm[:, cell_off[ca] : cell_off[ca] + cw] = cp[0:64, off : off + cw]
            if cb is not None:
                cm[:, cell_off[cb] : cell_off[cb] + cw] = (
                    cp[64:128, off : off + cw])
        rows = rows_all[c]
        real = rows != PAD
        gcell = rows[real] + c * CELLS_PER_CORE
        b_core = c // (N_CORES // B)
        xy = gcell - b_core * (GX * GY)
        out[b_core][:, xy] = cm[:, real]
    out = out.reshape(B, 64, GX, GY)
    if _trace:
        return out, res
    return out
